# revision 16
# baseline (speedup 1.0000x reference)
"""Trainium2 Bass kernel for nn_MessageAggregator (gnn_message_passing).

Computation (reference):
    s   = logsig(logsig(state @ W1_m.T + b1_m) @ W2_m.T)      # [E, D]
    agg = mask_transpose @ (mask @ s) - s                     # [E, D]
    out = logsig(logsig([agg, feature] @ W1_a.T + b1_a) @ W2_a.T)

Sharding: edge dimension E=32768 split across 8 cores (4096 edges each).
Each core:
  phase 0: memory-MLP on its edge slice (feature-major via PE transposes)
  phase 1: partial per-node aggregate  v = -(s.T @ mT_slice)  [D, N]
  AllReduce(v) over the 8 cores
  phase 2: edge aggregate  -(v.T)@mask_slice, subtract -s.T, concat-MLP,
           transpose-free edge-major final matmul, DMA out.

All matmuls run as float32r (fp32 bits, round-robin PE feed, full rate at
moving free dim >= 256).  log_sigmoid(x) = -softplus(-x) is computed
overflow-safely as softplus(t) = max(t,0) + ln(1 + exp(-|t|)) using the
Exp+Ln ACT table (z-values here reach +-5000, so exp(t) would overflow).
Sign bookkeeping keeps intermediates negated (u = -h) so each activation
is a single softplus; weight matrices are transposed/negated on device.
"""

import numpy as np

N_CORES = 8
E, N, D, DF = 32768, 2048, 128, 32
EL = E // N_CORES          # 4096 edges per core
NT = EL // 128             # 32 edge tiles of 128
NCH = EL // 512            # 8 chunks of 512 edges
P = 128

_CACHE: dict = {}


def _build():
    from concourse import bacc, mybir, tile

    F32 = mybir.dt.float32
    F32R = mybir.dt.float32r
    AF = mybir.ActivationFunctionType
    ALU = mybir.AluOpType

    nc = bacc.Bacc("TRN2", target_bir_lowering=False, debug=False,
                   num_devices=N_CORES)

    stateT_l = nc.dram_tensor("stateT_l", [D, EL], F32, kind="ExternalInput")
    featT_l = nc.dram_tensor("featT_l", [DF, EL], F32, kind="ExternalInput")
    mT_l = nc.dram_tensor("mT_l", [EL, N], F32, kind="ExternalInput")
    mask_l = nc.dram_tensor("mask_l", [N, EL], F32, kind="ExternalInput")
    w1m = nc.dram_tensor("w1m", [D, D], F32, kind="ExternalInput")
    b1m = nc.dram_tensor("b1m", [D], F32, kind="ExternalInput")
    w2m = nc.dram_tensor("w2m", [D, D], F32, kind="ExternalInput")
    w1a = nc.dram_tensor("w1a", [D, D + DF], F32, kind="ExternalInput")
    b1a = nc.dram_tensor("b1a", [D], F32, kind="ExternalInput")
    w2a = nc.dram_tensor("w2a", [D, D], F32, kind="ExternalInput")
    idn = nc.dram_tensor("idn", [P, P], F32, kind="ExternalInput")
    out_l = nc.dram_tensor("out_l", [EL, D], F32, kind="ExternalOutput")

    with tile.TileContext(nc) as tc:
        with (
            tc.tile_pool(name="consts", bufs=1) as consts,
            tc.tile_pool(name="persist", bufs=1) as persist,
            tc.tile_pool(name="tmp", bufs=2) as tmp,
            tc.tile_pool(name="mtp", bufs=16) as mtp,
            tc.tile_pool(name="maskp", bufs=20) as maskp,
            tc.tile_pool(name="outp", bufs=2) as outp,
            tc.tile_pool(name="ps_acc", bufs=1, space="PSUM") as ps_acc,
            tc.tile_pool(name="ps_mm", bufs=2, space="PSUM") as ps_mm,
            tc.tile_pool(name="ps_tp", bufs=2, space="PSUM") as ps_tp,
            tc.tile_pool(name="dram", bufs=1, space="DRAM") as dram,
        ):
            # ---------------- constants & weight prep ----------------
            idn_sb = consts.tile([P, P], F32)
            nc.sync.dma_start(idn_sb[:], idn[:])
            w1m_raw = consts.tile([D, D], F32)
            nc.sync.dma_start(w1m_raw[:], w1m[:])
            w2m_raw = consts.tile([D, D], F32)
            nc.sync.dma_start(w2m_raw[:], w2m[:])
            w1a_raw = consts.tile([D, D + DF], F32)
            nc.sync.dma_start(w1a_raw[:], w1a[:])
            w2a_raw = consts.tile([D, D], F32)
            nc.sync.dma_start(w2a_raw[:], w2a[:])
            b1m_sb = consts.tile([D, 1], F32)
            nc.sync.dma_start(b1m_sb[:], b1m[:, None])
            b1a_sb = consts.tile([D, 1], F32)
            nc.sync.dma_start(b1a_sb[:], b1a[:, None])

            tpw = ps_tp.tile([P, 512], F32, tag="tp")
            nc.tensor.transpose(tpw[:, 0:128], w1m_raw[:], idn_sb[:])
            nc.tensor.transpose(tpw[:, 128:256], w2m_raw[:], idn_sb[:])
            nc.tensor.transpose(tpw[:, 256:384], w1a_raw[:, 0:D], idn_sb[:])
            nc.tensor.transpose(tpw[:, 384:512], w2a_raw[:], idn_sb[:])
            w1mT = consts.tile([D, D], F32R)       # W1m.T
            nc.vector.tensor_copy(w1mT[:], tpw[:, 0:128])
            w2mnT = consts.tile([D, D], F32R)      # -(W2m.T)
            nc.vector.tensor_scalar_mul(w2mnT[:], tpw[:, 128:256], -1.0)
            w1anT = consts.tile([D, D], F32R)      # -(W1a[:, :D].T)
            nc.vector.tensor_scalar_mul(w1anT[:], tpw[:, 256:384], -1.0)
            w2anT = consts.tile([D, D], F32R)      # -(W2a.T)
            nc.vector.tensor_scalar_mul(w2anT[:], tpw[:, 384:512], -1.0)
            tpw2 = ps_tp.tile([P, 512], F32, tag="tp")
            nc.tensor.transpose(tpw2[:DF, 0:128], w1a_raw[:, D:], idn_sb[:])
            wa2T = consts.tile([DF, D], F32R)      # W1a[:, D:].T
            nc.vector.tensor_copy(wa2T[:], tpw2[:DF, 0:128])
            idn_bf = consts.tile([P, P], mybir.dt.bfloat16)
            nc.vector.tensor_copy(idn_bf[:], idn_sb[:])

            # ---------------- persistent intermediates ----------------
            u2T = persist.tile([P, EL], F32)           # -s.T  (feature-major)
            u2e = persist.tile([P, NT, D], F32R)       # -s    (edge-major tiles)
            featT = persist.tile([DF, EL], F32R)       # feature.T
            vT = persist.tile([P, N // P, D], F32R)    # -agg   [n, da] tiles

            stateT_sb = persist.tile([P, EL], F32R)
            for q4 in range(4):
                nc.sync.dma_start(
                    stateT_sb[:, q4 * 1024 : (q4 + 1) * 1024],
                    stateT_l[:, q4 * 1024 : (q4 + 1) * 1024].bitcast(F32R),
                )
            nc.sync.dma_start(featT[:], featT_l[:].bitcast(F32R))

            def softplus(z_ps, bias_ap, out_ap, w=512):
                """out = softplus(-z_ps - bias): 3 DVE + 2 ACT, overflow-safe."""
                t = tmp.tile([P, w], F32, tag="t")
                a = tmp.tile([P, w], F32, tag="a")
                if bias_ap is not None:
                    nc.vector.tensor_scalar(
                        t[:], z_ps, -1.0, bias_ap, ALU.mult, ALU.subtract
                    )
                else:
                    nc.vector.tensor_scalar_mul(t[:], z_ps, -1.0)
                nc.vector.tensor_scalar(
                    a[:].bitcast(mybir.dt.uint32),
                    t[:].bitcast(mybir.dt.uint32),
                    0x7FFFFFFF, None, ALU.bitwise_and,
                )
                ex = tmp.tile([P, w], F32, tag="ex")
                nc.scalar.activation(ex[:], a[:], AF.Exp, scale=-1.0)
                ln = tmp.tile([P, w], F32, tag="ln")
                nc.scalar.activation(ln[:], ex[:], AF.Ln, bias=1.0)
                nc.vector.scalar_tensor_tensor(
                    out_ap, t[:], 0.0, ln[:], ALU.max, ALU.add
                )

            # negated bias for the direct 2-ACT softplus in phase 0
            nb1m_sb = consts.tile([D, 1], F32)
            nc.vector.tensor_scalar_mul(nb1m_sb[:], b1m_sb[:], -1.0)

            # ------- phase 0 (memory MLP) interleaved with phase 1 -------
            # |z| <= ~4 in the memory MLP, so softplus(-z) = Ln(Exp(-z)+1)
            # directly (no overflow guard needed).  Phase-1 accumulators:
            # acc0/acc1 = node cols 0:1024, acc2/acc3 = 1024:2048.
            accs = [
                ps_acc.tile([P, 512], F32, tag=f"acc{q}", name=f"p1acc{q}")
                for q in range(4)
            ]
            for j in range(NCH):
                h1 = ps_mm.tile([P, 512], F32, tag="mm")
                nc.tensor.matmul(
                    h1[:], w1mT[:], stateT_sb[:, j * 512 : (j + 1) * 512],
                    start=True, stop=True,
                )
                ex1 = tmp.tile([P, 512], F32, tag="ex")
                nc.scalar.activation(ex1[:], h1[:], AF.Exp, scale=-1.0,
                                     bias=nb1m_sb[:])
                u1 = tmp.tile([P, 512], F32R, tag="u1")
                nc.scalar.activation(u1[:], ex1[:], AF.Ln, bias=1.0)
                z2 = ps_mm.tile([P, 512], F32, tag="mm")
                nc.tensor.matmul(z2[:], w2mnT[:], u1[:], start=True, stop=True)
                ex2 = tmp.tile([P, 512], F32, tag="ex")
                nc.scalar.activation(ex2[:], z2[:], AF.Exp, scale=-1.0)
                nc.scalar.activation(
                    u2T[:, j * 512 : (j + 1) * 512], ex2[:], AF.Ln, bias=1.0
                )

                tp2 = ps_tp.tile([P, 512], F32, tag="tp")
                for k in range(4):
                    c0 = (j * 4 + k) * P
                    nc.tensor.transpose(
                        tp2[:, k * P : (k + 1) * P],
                        u2T[:, c0 : c0 + P],
                        idn_sb[:],
                    )
                nc.vector.tensor_copy(
                    u2e[:, j * 4 : (j + 1) * 4, :].rearrange("p a d -> p (a d)"),
                    tp2[:],
                )

                # phase-1 matmuls for this chunk's 4 edge tiles; one
                # [128,512] tile per (e-tile, n-chunk) so 16 DMAs run
                # concurrently across queues
                for t_i in range(4 * j, 4 * j + 4):
                    for c4 in range(4):
                        mt = mtp.tile([P, 512], F32R, tag="mt",
                                      name=f"mt_{t_i}_{c4}")
                        nc.sync.dma_start(
                            mt[:],
                            mT_l[
                                t_i * P : (t_i + 1) * P,
                                c4 * 512 : (c4 + 1) * 512,
                            ].bitcast(F32R),
                        )
                        nc.tensor.matmul(
                            accs[c4][:],
                            u2e[:, t_i, :],
                            mt[:],
                            start=(t_i == 0),
                            stop=(t_i == NT - 1),
                        )

            # ---------------- AllReduce (per node-half) ----------------
            cc_outs = {}
            for nh in range(2):
                vsb = persist.tile([P, N // 2], mybir.dt.bfloat16, tag="vsb",
                                   name=f"vsb_{nh}", bufs=2)
                for q in range(2):
                    nc.vector.tensor_copy(
                        vsb[:, q * 512 : (q + 1) * 512], accs[2 * nh + q][:]
                    )
                cc_in = dram.tile([P, N // 2], mybir.dt.bfloat16,
                                  name=f"cc_in_{nh}")
                cc_out = dram.tile([P, N // 2], mybir.dt.bfloat16,
                                   addr_space="Shared", name=f"cc_out_{nh}")
                cc_outs[nh] = cc_out
                for hv in range(2):
                    nc.gpsimd.dma_start(
                        cc_in[:, hv * 512 : (hv + 1) * 512],
                        vsb[:, hv * 512 : (hv + 1) * 512],
                    )
                nc.gpsimd.collective_compute(
                    "AllReduce",
                    mybir.AluOpType.add,
                    ins=[cc_in.opt()],
                    outs=[cc_out.opt()],
                    replica_groups=[list(range(N_CORES))],
                )

            vfulls = {}
            for nh in range(2):
                vf = persist.tile([P, N // 2], mybir.dt.bfloat16,
                                  name=f"vfull_{nh}")
                vfulls[nh] = vf
                for hv in range(2):
                    nc.gpsimd.dma_start(
                        vf[:, hv * 512 : (hv + 1) * 512],
                        cc_outs[nh][:, hv * 512 : (hv + 1) * 512],
                    )

            def vt_transposes(nh):
                for g in range(2):
                    tp3 = ps_tp.tile([P, 512], mybir.dt.bfloat16, tag="tp",
                                     name=f"tp3_{nh}_{g}")
                    for k in range(4):
                        i = g * 4 + k
                        nc.tensor.transpose(
                            tp3[:, k * P : (k + 1) * P],
                            vfulls[nh][:, i * P : (i + 1) * P],
                            idn_bf[:],
                        )
                    blk = nh * 8 + g * 4
                    nc.vector.tensor_copy(
                        vT[:, blk : blk + 4, :].rearrange("p a d -> p (a d)"),
                        tp3[:],
                    )

            vt_transposes(0)

            # ---------------- phase 2: edge agg + concat MLP ----------------
            out_v = out_l.rearrange("(c k p) d -> c p k d", k=4, p=P)

            def p2_mask_mm(j, acc, nch_range):
                for nch in nch_range:
                    mk = maskp.tile([P, 512], F32R, tag="mk",
                                    name=f"mk_{j}_{nch}")
                    nc.sync.dma_start(
                        mk[:],
                        mask_l[
                            nch * P : (nch + 1) * P, j * 512 : (j + 1) * 512
                        ].bitcast(F32R),
                    )
                    nc.tensor.matmul(
                        acc[:],
                        vT[:, nch, :],
                        mk[:],
                        start=(nch == 0),
                        stop=(nch == N // P - 1),
                    )

            def p2_mlp(j, acc):
                w3 = tmp.tile([P, 512], F32R, tag="w3", name=f"w3_{j}")
                nc.vector.tensor_sub(
                    w3[:], acc[:], u2T[:, j * 512 : (j + 1) * 512]
                )
                z1a = ps_mm.tile([P, 512], F32, tag="mm", name=f"z1a_{j}")
                nc.tensor.matmul(z1a[:], w1anT[:], w3[:], start=True, stop=False)
                nc.tensor.matmul(
                    z1a[:],
                    wa2T[:],
                    featT[:, j * 512 : (j + 1) * 512],
                    start=False,
                    stop=True,
                )
                u3 = tmp.tile([P, 512], F32R, tag="u3", name=f"u3_{j}")
                softplus(z1a[:], b1a_sb[:], u3[:])
                po = ps_mm.tile([P, 512], F32, tag="mm", name=f"po_{j}")
                for k in range(4):
                    nc.tensor.matmul(
                        po[:, k * P : (k + 1) * P],
                        u3[:, k * P : (k + 1) * P],
                        w2anT[:],
                        start=True,
                        stop=True,
                    )
                # out = logsig(z2a) = min(po,0) - ln(1+exp(-|po|)), po=z2a
                a2 = tmp.tile([P, 512], F32, tag="a", name=f"a2_{j}")
                nc.vector.tensor_scalar(
                    a2[:].bitcast(mybir.dt.uint32),
                    po[:].bitcast(mybir.dt.uint32),
                    0x7FFFFFFF, None, ALU.bitwise_and,
                )
                e2 = tmp.tile([P, 512], F32, tag="ex", name=f"e2_{j}")
                nc.scalar.activation(e2[:], a2[:], AF.Exp, scale=-1.0)
                l2 = tmp.tile([P, 512], F32, tag="ln", name=f"l2_{j}")
                nc.scalar.activation(l2[:], e2[:], AF.Ln, bias=1.0)
                ob = outp.tile([P, 512], F32, tag="ob", name=f"ob_{j}")
                nc.vector.scalar_tensor_tensor(
                    ob[:], po[:], 0.0, l2[:], ALU.min, ALU.subtract
                )
                nc.gpsimd.dma_start(
                    out_v[j], ob.rearrange("p (k d) -> p k d", k=4)
                )

            # chunks 0-3: node-half A matmuls first (only need vT blocks 0-7),
            # then half-B transposes, then the rest.
            first = [
                ps_acc.tile([P, 512], F32, tag=f"acc{q}", name=f"p2acc_{q}")
                for q in range(4)
            ]
            for j in range(4):
                p2_mask_mm(j, first[j], range(8))
            vt_transposes(1)
            for j in range(4):
                p2_mask_mm(j, first[j], range(8, 16))
                p2_mlp(j, first[j])
            for j in range(4, NCH):
                acc = ps_acc.tile([P, 512], F32, tag=f"acc{j % 4}",
                                  name=f"p2acc2_{j}")
                p2_mask_mm(j, acc, range(16))
                p2_mlp(j, acc)

    nc.compile()
    return nc


def kernel(**inputs: np.ndarray) -> np.ndarray:
    from concourse.bass_utils import run_bass_kernel_spmd

    if "nc" not in _CACHE:
        _CACHE["nc"] = _build()
    nc = _CACHE["nc"]

    state = np.ascontiguousarray(inputs["state"], dtype=np.float32)
    feature = np.ascontiguousarray(inputs["feature"], dtype=np.float32)
    mask = np.ascontiguousarray(inputs["mask"], dtype=np.float32)
    mask_transpose = np.ascontiguousarray(
        inputs["mask_transpose"], dtype=np.float32
    )
    idn_np = np.eye(P, dtype=np.float32)

    common = {
        "w1m": np.ascontiguousarray(inputs["W1_m"], dtype=np.float32),
        "b1m": np.ascontiguousarray(inputs["b1_m"], dtype=np.float32),
        "w2m": np.ascontiguousarray(inputs["W2_m"], dtype=np.float32),
        "w1a": np.ascontiguousarray(inputs["W1_a"], dtype=np.float32),
        "b1a": np.ascontiguousarray(inputs["b1_a"], dtype=np.float32),
        "w2a": np.ascontiguousarray(inputs["W2_a"], dtype=np.float32),
        "idn": idn_np,
    }
    in_maps = []
    for c in range(N_CORES):
        sl = slice(c * EL, (c + 1) * EL)
        in_maps.append(
            {
                "stateT_l": np.ascontiguousarray(state[sl].T),
                "featT_l": np.ascontiguousarray(feature[sl].T),
                "mT_l": mask_transpose[sl],
                "mask_l": np.ascontiguousarray(mask[:, sl]),
                **common,
            }
        )
    _CACHE["in_maps"] = in_maps

    res = run_bass_kernel_spmd(nc, in_maps, core_ids=list(range(N_CORES)))
    out = np.concatenate(
        [res.results[c]["out_l"] for c in range(N_CORES)], axis=0
    )
    return out


# revision 18
# speedup vs baseline: 1.0684x; 1.0684x over previous
"""Trainium2 Bass kernel for nn_MessageAggregator (gnn_message_passing).

Computation (reference):
    s   = logsig(logsig(state @ W1_m.T + b1_m) @ W2_m.T)      # [E, D]
    agg = mask_transpose @ (mask @ s) - s                     # [E, D]
    out = logsig(logsig([agg, feature] @ W1_a.T + b1_a) @ W2_a.T)

Sharding: edge dimension E=32768 split across 8 cores (4096 edges each).
Each core:
  phase 0: memory-MLP on its edge slice (feature-major via PE transposes)
  phase 1: partial per-node aggregate  v = -(s.T @ mT_slice)  [D, N]
  AllReduce(v) over the 8 cores
  phase 2: edge aggregate  -(v.T)@mask_slice, subtract -s.T, concat-MLP,
           transpose-free edge-major final matmul, DMA out.

All matmuls run as float32r (fp32 bits, round-robin PE feed, full rate at
moving free dim >= 256).  log_sigmoid(x) = -softplus(-x) is computed
overflow-safely as softplus(t) = max(t,0) + ln(1 + exp(-|t|)) using the
Exp+Ln ACT table (z-values here reach +-5000, so exp(t) would overflow).
Sign bookkeeping keeps intermediates negated (u = -h) so each activation
is a single softplus; weight matrices are transposed/negated on device.
"""

import ml_dtypes
import numpy as np

N_CORES = 8
E, N, D, DF = 32768, 2048, 128, 32
EL = E // N_CORES          # 4096 edges per core
NT = EL // 128             # 32 edge tiles of 128
NCH = EL // 512            # 8 chunks of 512 edges
P = 128

_CACHE: dict = {}


def _build():
    from concourse import bacc, mybir, tile

    F32 = mybir.dt.float32
    F32R = mybir.dt.float32r
    AF = mybir.ActivationFunctionType
    ALU = mybir.AluOpType

    nc = bacc.Bacc("TRN2", target_bir_lowering=False, debug=False,
                   num_devices=N_CORES)

    stateT_l = nc.dram_tensor("stateT_l", [D, EL], mybir.dt.bfloat16, kind="ExternalInput")
    featT_l = nc.dram_tensor("featT_l", [DF, EL], mybir.dt.bfloat16, kind="ExternalInput")
    mT_l = nc.dram_tensor("mT_l", [EL, N], F32, kind="ExternalInput")
    mask_l = nc.dram_tensor("mask_l", [N, EL], F32, kind="ExternalInput")
    w1m = nc.dram_tensor("w1m", [D, D], F32, kind="ExternalInput")
    b1m = nc.dram_tensor("b1m", [D], F32, kind="ExternalInput")
    w2m = nc.dram_tensor("w2m", [D, D], F32, kind="ExternalInput")
    w1a = nc.dram_tensor("w1a", [D, D + DF], F32, kind="ExternalInput")
    b1a = nc.dram_tensor("b1a", [D], F32, kind="ExternalInput")
    w2a = nc.dram_tensor("w2a", [D, D], F32, kind="ExternalInput")
    idn = nc.dram_tensor("idn", [P, P], F32, kind="ExternalInput")
    out_l = nc.dram_tensor("out_l", [EL, D], F32, kind="ExternalOutput")

    with tile.TileContext(nc) as tc:
        with (
            tc.tile_pool(name="consts", bufs=1) as consts,
            tc.tile_pool(name="persist", bufs=1) as persist,
            tc.tile_pool(name="tmp", bufs=2) as tmp,
            tc.tile_pool(name="mtp", bufs=20) as mtp,
            tc.tile_pool(name="maskp", bufs=24) as maskp,
            tc.tile_pool(name="outp", bufs=2) as outp,
            tc.tile_pool(name="ps_acc", bufs=1, space="PSUM") as ps_acc,
            tc.tile_pool(name="ps_mm", bufs=2, space="PSUM") as ps_mm,
            tc.tile_pool(name="ps_tp", bufs=2, space="PSUM") as ps_tp,
            tc.tile_pool(name="dram", bufs=1, space="DRAM") as dram,
        ):
            # ---------------- constants & weight prep ----------------
            idn_sb = consts.tile([P, P], F32)
            nc.sync.dma_start(idn_sb[:], idn[:])
            w1m_raw = consts.tile([D, D], F32)
            nc.sync.dma_start(w1m_raw[:], w1m[:])
            w2m_raw = consts.tile([D, D], F32)
            nc.sync.dma_start(w2m_raw[:], w2m[:])
            w1a_raw = consts.tile([D, D + DF], F32)
            nc.sync.dma_start(w1a_raw[:], w1a[:])
            w2a_raw = consts.tile([D, D], F32)
            nc.sync.dma_start(w2a_raw[:], w2a[:])
            b1m_sb = consts.tile([D, 1], F32)
            nc.sync.dma_start(b1m_sb[:], b1m[:, None])
            b1a_sb = consts.tile([D, 1], F32)
            nc.sync.dma_start(b1a_sb[:], b1a[:, None])

            tpw = ps_tp.tile([P, 512], F32, tag="tp")
            nc.tensor.transpose(tpw[:, 0:128], w1m_raw[:], idn_sb[:])
            nc.tensor.transpose(tpw[:, 128:256], w2m_raw[:], idn_sb[:])
            nc.tensor.transpose(tpw[:, 256:384], w1a_raw[:, 0:D], idn_sb[:])
            nc.tensor.transpose(tpw[:, 384:512], w2a_raw[:], idn_sb[:])
            w1mT = consts.tile([D, D], mybir.dt.bfloat16)       # W1m.T
            nc.vector.tensor_copy(w1mT[:], tpw[:, 0:128])
            w2mnT = consts.tile([D, D], F32R)      # -(W2m.T)
            nc.vector.tensor_scalar_mul(w2mnT[:], tpw[:, 128:256], -1.0)
            w1anT = consts.tile([D, D], F32R)      # -(W1a[:, :D].T)
            nc.vector.tensor_scalar_mul(w1anT[:], tpw[:, 256:384], -1.0)
            w2anT = consts.tile([D, D], F32R)      # -(W2a.T)
            nc.vector.tensor_scalar_mul(w2anT[:], tpw[:, 384:512], -1.0)
            tpw2 = ps_tp.tile([P, 512], F32, tag="tp")
            nc.tensor.transpose(tpw2[:DF, 0:128], w1a_raw[:, D:], idn_sb[:])
            wa2T = consts.tile([DF, D], mybir.dt.bfloat16)  # W1a[:, D:].T
            nc.vector.tensor_copy(wa2T[:], tpw2[:DF, 0:128])
            idn_bf = consts.tile([P, P], mybir.dt.bfloat16)
            nc.vector.tensor_copy(idn_bf[:], idn_sb[:])

            # ---------------- persistent intermediates ----------------
            u2T = persist.tile([P, EL], mybir.dt.bfloat16)  # -s.T (feat-major)
            u2e = persist.tile([P, NT, D], F32R)       # -s    (edge-major tiles)
            featT = persist.tile([DF, EL], mybir.dt.bfloat16)  # feature.T
            vT = persist.tile([P, N // P, D], F32R)    # -agg   [n, da] tiles

            stateT_sb = persist.tile([P, EL], mybir.dt.bfloat16)
            for q4 in range(4):
                nc.sync.dma_start(
                    stateT_sb[:, q4 * 1024 : (q4 + 1) * 1024],
                    stateT_l[:, q4 * 1024 : (q4 + 1) * 1024],
                )
            nc.sync.dma_start(featT[:], featT_l[:])

            def softplus(z_ps, bias_ap, out_ap, w=512):
                """out = softplus(-z_ps - bias): 3 DVE + 2 ACT, overflow-safe."""
                t = tmp.tile([P, w], F32, tag="t")
                a = tmp.tile([P, w], F32, tag="a")
                if bias_ap is not None:
                    nc.vector.tensor_scalar(
                        t[:], z_ps, -1.0, bias_ap, ALU.mult, ALU.subtract
                    )
                else:
                    nc.vector.tensor_scalar_mul(t[:], z_ps, -1.0)
                nc.vector.tensor_scalar(
                    a[:].bitcast(mybir.dt.uint32),
                    t[:].bitcast(mybir.dt.uint32),
                    0x7FFFFFFF, None, ALU.bitwise_and,
                )
                ex = tmp.tile([P, w], F32, tag="ex")
                nc.scalar.activation(ex[:], a[:], AF.Exp, scale=-1.0)
                ln = tmp.tile([P, w], F32, tag="ln")
                nc.scalar.activation(ln[:], ex[:], AF.Ln, bias=1.0)
                nc.vector.scalar_tensor_tensor(
                    out_ap, t[:], 0.0, ln[:], ALU.max, ALU.add
                )

            # negated bias for the direct 2-ACT softplus in phase 0
            nb1m_sb = consts.tile([D, 1], F32)
            nc.vector.tensor_scalar_mul(nb1m_sb[:], b1m_sb[:], -1.0)

            # ------- phase 0 (memory MLP) interleaved with phase 1 -------
            # |z| <= ~4 in the memory MLP, so softplus(-z) = Ln(Exp(-z)+1)
            # directly (no overflow guard needed).  Phase-1 accumulators:
            # acc0/acc1 = node cols 0:1024, acc2/acc3 = 1024:2048.
            accs = [
                ps_acc.tile([P, 512], F32, tag=f"acc{q}", name=f"p1acc{q}")
                for q in range(4)
            ]
            for j in range(NCH):
                h1 = ps_mm.tile([P, 512], F32, tag="mm")
                nc.tensor.matmul(
                    h1[:], w1mT[:], stateT_sb[:, j * 512 : (j + 1) * 512],
                    start=True, stop=True,
                )
                ex1 = tmp.tile([P, 512], F32, tag="ex")
                nc.scalar.activation(ex1[:], h1[:], AF.Exp, scale=-1.0,
                                     bias=nb1m_sb[:])
                u1 = tmp.tile([P, 512], F32R, tag="u1")
                nc.scalar.activation(u1[:], ex1[:], AF.Ln, bias=1.0)
                z2 = ps_mm.tile([P, 512], F32, tag="mm")
                nc.tensor.matmul(z2[:], w2mnT[:], u1[:], start=True, stop=True)
                ex2 = tmp.tile([P, 512], F32, tag="ex")
                nc.scalar.activation(ex2[:], z2[:], AF.Exp, scale=-1.0)
                nc.scalar.activation(
                    u2T[:, j * 512 : (j + 1) * 512], ex2[:], AF.Ln, bias=1.0
                )

                tp2 = ps_tp.tile([P, 512], mybir.dt.bfloat16, tag="tp",
                                 name=f"tp2_{j}")
                for k in range(4):
                    c0 = (j * 4 + k) * P
                    nc.tensor.transpose(
                        tp2[:, k * P : (k + 1) * P],
                        u2T[:, c0 : c0 + P],
                        idn_bf[:],
                    )
                nc.vector.tensor_copy(
                    u2e[:, j * 4 : (j + 1) * 4, :].rearrange("p a d -> p (a d)"),
                    tp2[:],
                )

                # phase-1 matmuls for this chunk's 4 edge tiles; one
                # [128,512] tile per (e-tile, n-chunk) so 16 DMAs run
                # concurrently across queues
                for t_i in range(4 * j, 4 * j + 4):
                    for c4 in range(4):
                        mt = mtp.tile([P, 512], F32R, tag="mt",
                                      name=f"mt_{t_i}_{c4}")
                        nc.sync.dma_start(
                            mt[:],
                            mT_l[
                                t_i * P : (t_i + 1) * P,
                                c4 * 512 : (c4 + 1) * 512,
                            ].bitcast(F32R),
                        )
                        nc.tensor.matmul(
                            accs[c4][:],
                            u2e[:, t_i, :],
                            mt[:],
                            start=(t_i == 0),
                            stop=(t_i == NT - 1),
                        )

            # ---------------- AllReduce (single, bf16) ----------------
            vsb = persist.tile([P, N], mybir.dt.bfloat16)
            for q in range(4):
                nc.vector.tensor_copy(
                    vsb[:, q * 512 : (q + 1) * 512], accs[q][:]
                )
            cc_in = dram.tile([P, N], mybir.dt.bfloat16)
            cc_out = dram.tile([P, N], mybir.dt.bfloat16, addr_space="Shared")
            for hv in range(4):
                nc.gpsimd.dma_start(
                    cc_in[:, hv * 512 : (hv + 1) * 512],
                    vsb[:, hv * 512 : (hv + 1) * 512],
                )
            nc.gpsimd.collective_compute(
                "AllReduce",
                mybir.AluOpType.add,
                ins=[cc_in.opt()],
                outs=[cc_out.opt()],
                replica_groups=[list(range(N_CORES))],
            )
            vfull = persist.tile([P, N], mybir.dt.bfloat16)
            for hv in range(4):
                nc.gpsimd.dma_start(
                    vfull[:, hv * 512 : (hv + 1) * 512],
                    cc_out[:, hv * 512 : (hv + 1) * 512],
                )

            for g in range(4):
                tp3 = ps_tp.tile([P, 512], mybir.dt.bfloat16, tag="tp",
                                 name=f"tp3_{g}")
                for k in range(4):
                    i = g * 4 + k
                    nc.tensor.transpose(
                        tp3[:, k * P : (k + 1) * P],
                        vfull[:, i * P : (i + 1) * P],
                        idn_bf[:],
                    )
                nc.vector.tensor_copy(
                    vT[:, g * 4 : (g + 1) * 4, :].rearrange("p a d -> p (a d)"),
                    tp3[:],
                )

            # ---------------- phase 2: edge agg + concat MLP ----------------
            out_v = out_l.rearrange("(c k p) d -> c p k d", k=4, p=P)

            def p2_mask_mm(j, acc, nch_range):
                for nch in nch_range:
                    mk = maskp.tile([P, 512], F32R, tag="mk",
                                    name=f"mk_{j}_{nch}")
                    nc.sync.dma_start(
                        mk[:],
                        mask_l[
                            nch * P : (nch + 1) * P, j * 512 : (j + 1) * 512
                        ].bitcast(F32R),
                    )
                    nc.tensor.matmul(
                        acc[:],
                        vT[:, nch, :],
                        mk[:],
                        start=(nch == 0),
                        stop=(nch == N // P - 1),
                    )

            def p2_mlp(j, acc):
                w3 = tmp.tile([P, 512], F32R, tag="w3", name=f"w3_{j}")
                nc.vector.tensor_sub(
                    w3[:], acc[:], u2T[:, j * 512 : (j + 1) * 512]
                )
                z1a = ps_mm.tile([P, 512], F32, tag="mm", name=f"z1a_{j}")
                nc.tensor.matmul(z1a[:], w1anT[:], w3[:], start=True, stop=False)
                nc.tensor.matmul(
                    z1a[:],
                    wa2T[:],
                    featT[:, j * 512 : (j + 1) * 512],
                    start=False,
                    stop=True,
                )
                u3 = tmp.tile([P, 512], F32R, tag="u3", name=f"u3_{j}")
                softplus(z1a[:], b1a_sb[:], u3[:])
                po = ps_mm.tile([P, 512], F32, tag="mm", name=f"po_{j}")
                for k in range(4):
                    nc.tensor.matmul(
                        po[:, k * P : (k + 1) * P],
                        u3[:, k * P : (k + 1) * P],
                        w2anT[:],
                        start=True,
                        stop=True,
                    )
                # out = logsig(z2a) = min(po,0) - ln(1+exp(-|po|)), po=z2a
                a2 = tmp.tile([P, 512], F32, tag="a", name=f"a2_{j}")
                nc.vector.tensor_scalar(
                    a2[:].bitcast(mybir.dt.uint32),
                    po[:].bitcast(mybir.dt.uint32),
                    0x7FFFFFFF, None, ALU.bitwise_and,
                )
                e2 = tmp.tile([P, 512], F32, tag="ex", name=f"e2_{j}")
                nc.scalar.activation(e2[:], a2[:], AF.Exp, scale=-1.0)
                l2 = tmp.tile([P, 512], F32, tag="ln", name=f"l2_{j}")
                nc.scalar.activation(l2[:], e2[:], AF.Ln, bias=1.0)
                ob = outp.tile([P, 512], F32, tag="ob", name=f"ob_{j}")
                nc.vector.scalar_tensor_tensor(
                    ob[:], po[:], 0.0, l2[:], ALU.min, ALU.subtract
                )
                nc.gpsimd.dma_start(
                    out_v[j], ob.rearrange("p (k d) -> p k d", k=4)
                )

            for j in range(NCH):
                acc = ps_acc.tile([P, 512], F32, tag=f"acc{j % 4}",
                                  name=f"p2acc_{j}")
                p2_mask_mm(j, acc, range(16))
                p2_mlp(j, acc)

    nc.compile()
    return nc


def kernel(**inputs: np.ndarray) -> np.ndarray:
    from concourse.bass_utils import run_bass_kernel_spmd

    if "nc" not in _CACHE:
        _CACHE["nc"] = _build()
    nc = _CACHE["nc"]

    state = np.ascontiguousarray(inputs["state"], dtype=np.float32)
    feature = np.ascontiguousarray(inputs["feature"], dtype=np.float32)
    mask = np.ascontiguousarray(inputs["mask"], dtype=np.float32)
    mask_transpose = np.ascontiguousarray(
        inputs["mask_transpose"], dtype=np.float32
    )
    idn_np = np.eye(P, dtype=np.float32)

    common = {
        "w1m": np.ascontiguousarray(inputs["W1_m"], dtype=np.float32),
        "b1m": np.ascontiguousarray(inputs["b1_m"], dtype=np.float32),
        "w2m": np.ascontiguousarray(inputs["W2_m"], dtype=np.float32),
        "w1a": np.ascontiguousarray(inputs["W1_a"], dtype=np.float32),
        "b1a": np.ascontiguousarray(inputs["b1_a"], dtype=np.float32),
        "w2a": np.ascontiguousarray(inputs["W2_a"], dtype=np.float32),
        "idn": idn_np,
    }
    in_maps = []
    for c in range(N_CORES):
        sl = slice(c * EL, (c + 1) * EL)
        in_maps.append(
            {
                "stateT_l": np.ascontiguousarray(state[sl].T).astype(
                    ml_dtypes.bfloat16
                ),
                "featT_l": np.ascontiguousarray(feature[sl].T).astype(
                    ml_dtypes.bfloat16
                ),
                "mT_l": mask_transpose[sl],
                "mask_l": np.ascontiguousarray(mask[:, sl]),
                **common,
            }
        )
    _CACHE["in_maps"] = in_maps

    res = run_bass_kernel_spmd(nc, in_maps, core_ids=list(range(N_CORES)))
    out = np.concatenate(
        [res.results[c]["out_l"] for c in range(N_CORES)], axis=0
    )
    return out
